# revision 18
# baseline (speedup 1.0000x reference)
"""AttnBlock (GroupNorm -> QKV -> 4096x4096 single-head attention -> proj ->
residual) on 8 TRN2 NeuronCores.

Sharding: data-parallel over batch (B=2) x sequence-parallel over query
positions (4 slabs of 1024). Each core receives the full x[b] (rolled so its
query slab sits at columns 0:1024), computes GroupNorm + v for the whole
image (replicated within the 4-core batch group -> zero collectives), and
attention + projection + residual for its 1024 query columns only.

Precision/structure highlights:
- All heavy matmuls are fp8e4m3 DoubleRow (2x128 contraction / instr at
  0.5 cyc/row), fp32 PSUM.
- k is never materialized: score = (Wk^T q)^T . h, so we compute
  u = Wk^T q_f8 (C x 1024) and use the fp8 x itself as the score
  stationary. The bk term is per-query constant -> softmax invariant ->
  dropped.
- GroupNorm folds: scale -> folded into wq/wv weight tiles on device
  (per-partition) and into the u drain; shift -> tiny N=1 bias-chain
  matmuls (bq2 = 32(Wq shift + bq)) plus a per-channel residual constant
  out += Wp (Wv shift + bv) handled via x_sl. GN stats use a stride-8
  subsample (var estimator err ~1.6%, far below fp8 noise).
- Softmax denominators via a DoubleRow matmul with constant-4.0
  stationary; the o drain fuses the normalization (tensor_tensor with the
  broadcast reciprocal).
"""
import sys
sys.path.insert(0, '/opt/trn_rl_repo')
import contextlib
import numpy as np
import ml_dtypes

import concourse.bass as bass
import concourse.tile as tile
from concourse import mybir, bacc
from concourse import bass_utils

f32 = mybir.dt.float32
bf16 = mybir.dt.bfloat16
fp8 = mybir.dt.float8e4
AF = mybir.ActivationFunctionType
ALU = mybir.AluOpType
DR = mybir.MatmulPerfMode.DoubleRow
F8 = ml_dtypes.float8_e4m3

C = 512          # channels
N = 4096         # positions (64*64)
G = 32           # groupnorm groups
GP = 16          # channels per group
NT = C // 128    # 4 channel partition-tiles
QS = 1024        # query slab per core
QC = 256         # query chunk in attention phase
EPS = 1e-6
SW = 32.0        # weight scale folded into all four fp8 weights
SEXP_LEGACY = 1.0 / (16.0 * float(np.sqrt(C)))
SEXP_FAST = 1.0 / (32.0 * float(np.sqrt(C)))
OSC = 2.0 ** -8  # final pp drain scale (pp = 256 * wp.o_norm)
STRIDE = 8       # GN stats subsample stride
NSAMP = float(GP * (N // STRIDE))        # samples per group


def _emit_body(nc, tc, p, x8_d, xsl_d, wall_d, misc_d, out, legacy_q):
    sb, scr, pq, outp, small = p["sb"], p["scr"], p["pq"], p["outp"], p["small"]
    ps_mm, ps_o, ps_sums, ps_sm = p["ps_mm"], p["ps_o"], p["ps_sums"], p["ps_sm"]

    ones_row = small.tile([1, 128], bf16, tag="ones_row")
    nc.vector.memset(ones_row[:], 1.0)
    ones4 = small.tile([128, 2, 32], fp8, tag="ones4")
    nc.vector.memset(ones4[:], 4.0)
    eps8 = small.tile([8, 1], f32, tag="eps8")
    nc.vector.memset(eps8[:], EPS)

    # ---- phase 0: DMA + GN stats ---------------------------------
    xr = x8_d.ap().rearrange("(t p) n -> p t n", p=128)
    x8a = sb.tile([128, 2, N], fp8, tag="x8a")
    nc.sync.dma_start(x8a[:], xr[:, 0:2, :])
    wall = sb.tile([128, 4, NT, C], fp8, tag="wall")
    nc.sync.dma_start(wall[:], wall_d.ap())
    misc = sb.tile([128, 672], f32, tag="misc")
    nc.sync.dma_start(misc[:], misc_d.ap())
    x8b = sb.tile([128, 2, N], fp8, tag="x8b")
    nc.sync.dma_start(x8b[:], xr[:, 2:4, :])
    xpair = [x8a, x8b]

    w_q, w_kT, w_v, w_p = (wall[:, i, :, :] for i in range(4))
    bq32 = [misc[:, t:t + 1] for t in range(NT)]          # 32*bq
    bv32 = [misc[:, 4 + t:5 + t] for t in range(NT)]      # 32*bv (cout tiles)
    bp_t = [misc[:, 8 + t:9 + t] for t in range(NT)]
    gnw_t = [misc[:, 12 + t:13 + t] for t in range(NT)]
    gnb_t = [misc[:, 16 + t:17 + t] for t in range(NT)]
    sel8p = misc[:, 20:148]        # [128,128] group selector, zero-padded
    sel8T = misc[0:8, 160:288]

    # stats on stride-8 subsample: sum on DVE, sumsq on ACT
    stats2 = small.tile([128, 8], f32, tag="stats2")      # cols 0:4 sum, 4:8 sq
    for t in range(NT):
        samp = xpair[t // 2][:, t % 2, :].rearrange(
            "p (n s) -> p s n", s=STRIDE)[:, 0, :]
        nc.vector.reduce_sum(stats2[:, t:t + 1], samp, axis=mybir.AxisListType.X)
        sqs = scr.tile([128, N // STRIDE], bf16, tag="sqs")
        nc.scalar.activation(sqs[:], samp, AF.Square,
                             accum_out=stats2[:, 4 + t:5 + t])

    # batched GN finalize: one Ln + one Exp for all 32 groups
    ps_all = ps_sm.tile([128, 512], f32, tag="sm")    # one bank for all
    # sel8p is zero-padded to 128 stationary columns so this first matmul
    # covers all 128 partitions: its start=True pending-zeroes the whole
    # bank for every later start=False matmul below.
    ps_g = ps_all[:, 0:8]
    nc.tensor.matmul(ps_g, sel8p, stats2[:], start=True, stop=True)
    mst = small.tile([8, 8], f32, tag="mst")
    nc.scalar.mul(mst[:], ps_g[0:8, :], 1.0 / NSAMP)           # mean t | ex2 t
    var4 = small.tile([8, 4], f32, tag="var4")
    m2 = small.tile([8, 4], f32, tag="m2")
    nc.vector.tensor_tensor(m2[:], mst[:, 0:4], mst[:, 0:4], op=ALU.mult)
    nc.vector.tensor_tensor(var4[:], mst[:, 4:8], m2[:], op=ALU.subtract)
    lnv = small.tile([8, 4], f32, tag="lnv")
    nc.scalar.activation(lnv[:], var4[:], AF.Ln, bias=eps8[:])
    grp2 = small.tile([8, 2, NT], f32, tag="grp2")        # mean row | rstd row
    nc.vector.tensor_copy(grp2[:, 0, :], mst[:, 0:4])
    nc.scalar.activation(grp2[:, 1, :], lnv[:], AF.Exp, scale=-0.5)
    # NOTE: ps_g's start=True zeroed this whole PSUM bank; every later
    # matmul into the ps_sm bank must use start=False (pending-zero) so it
    # does not clobber earlier results that are still being read.
    ps_bc = ps_all[:, 8:16]                               # mean t | rstd t
    nc.tensor.matmul(ps_bc, sel8T, grp2[:].rearrange("p a b -> p (a b)"),
                     start=False, stop=True)
    # scale/shift per tile + derived columns
    scale_t, sc6_t = [], []
    sh8 = small.tile([128, NT], fp8, tag="sh8")           # 128*shift, fp8
    for t in range(NT):
        sc = small.tile([128, 1], f32, tag=f"scale{t}")
        nc.vector.tensor_tensor(sc[:], gnw_t[t], ps_bc[:, 4 + t:5 + t],
                                op=ALU.mult)
        sc6 = small.tile([128, 1], f32, tag=f"sc6{t}")
        nc.vector.tensor_scalar_mul(sc6[:], sc[:], 2.0 ** -6)
        nsc = small.tile([128, 1], f32, tag=f"nscale{t}")
        nc.vector.tensor_scalar_mul(nsc[:], sc[:], -1.0)
        sh = small.tile([128, 1], f32, tag=f"shift{t}")
        nc.vector.scalar_tensor_tensor(sh[:], ps_bc[:, t:t + 1], nsc[:],
                                       gnb_t[t], op0=ALU.mult, op1=ALU.add)
        nc.vector.tensor_scalar_mul(sh8[:, t:t + 1], sh[:], 128.0)
        scale_t.append(sc)
        sc6_t.append(sc6)

    # bias chains (N=1 fp8 matmuls, before the in-place weight folds):
    # bq2 = 32(Wq shift + bq);  vc = 2^5 (Wv shift + bv);
    # badd = Wp vc / 2^10  (-> residual constant Wp(Wv shift + bv))
    ps_bq = ps_all[:, 16:20]
    ps_vc = ps_all[:, 20:24]
    for t in range(NT):
        for s in range(NT):
            nc.tensor.matmul(ps_bq[:, t:t + 1],
                             w_q[:, s, t * 128:(t + 1) * 128], sh8[:, s:s + 1],
                             start=False, stop=(s == 3))
            nc.tensor.matmul(ps_vc[:, t:t + 1],
                             w_v[:, s, t * 128:(t + 1) * 128], sh8[:, s:s + 1],
                             start=False, stop=(s == 3))
    if legacy_q:
        bq2 = small.tile([128, 4], f32, tag="bq2")
        for t in range(NT):
            nc.vector.tensor_scalar(bq2[:, t:t + 1], ps_bq[:, t:t + 1],
                                    2.0 ** -7, bq32[t], op0=ALU.mult,
                                    op1=ALU.add)
    else:
        # ub = scale o (M shift) * 32  (chain psum = 4096 * M shift)
        ub = small.tile([128, 4], f32, tag="ub")
        for t in range(NT):
            nc.vector.tensor_scalar(ub[:, t:t + 1], ps_bq[:, t:t + 1],
                                    scale_t[t][:], 2.0 ** -7, op0=ALU.mult,
                                    op1=ALU.mult)
    vc8 = small.tile([128, 4], fp8, tag="vc8")
    for t in range(NT):
        nc.scalar.activation(vc8[:, t:t + 1], ps_vc[:, t:t + 1], AF.Identity,
                             scale=2.0 ** -7, bias=bv32[t])
    ps_t2 = ps_all[:, 24:28]
    for t in range(NT):
        for s in range(NT):
            nc.tensor.matmul(ps_t2[:, t:t + 1],
                             w_p[:, s, t * 128:(t + 1) * 128], vc8[:, s:s + 1],
                             start=False, stop=(s == 3))
    badd = small.tile([128, 4], f32, tag="badd")
    for t in range(NT):
        nc.vector.tensor_scalar(badd[:, t:t + 1], ps_t2[:, t:t + 1],
                                2.0 ** -10, bp_t[t], op0=ALU.mult, op1=ALU.add)

    # fold GN scale into wq / wv (per-partition, in place, Pool)
    for s in range(NT):
        nc.gpsimd.tensor_scalar_mul(w_q[:, s, :], w_q[:, s, :], scale_t[s][:])
        nc.gpsimd.tensor_scalar_mul(w_v[:, s, :], w_v[:, s, :], scale_t[s][:])

    # residual slab + per-channel constant (bp + Wp(Wv shift + bv))
    x_sl = sb.tile([128, NT, QS], f32, tag="x_sl")
    nc.sync.dma_start(x_sl[:], xsl_d.ap().rearrange("(t p) n -> p t n", p=128))
    for t in range(NT):
        nc.gpsimd.tensor_scalar_add(x_sl[:, t, :], x_sl[:, t, :],
                                    badd[:, t:t + 1])

    # ---- phase 1: u (drains alternate ACT/DVE) -------------------
    # Fast path (bq == 0): host sends M = Wk^T Wq in the q slot, so
    # u = scale o (M (scale o x + shift)) comes straight from x with no q
    # projection; the shift term lands in the u-drain bias via the chain.
    # Legacy path (bq != 0): q = Wq h + bq is materialized, then
    # u = Wk^T q (the bq^T k score term is not softmax-invariant).
    u_f8 = sb.tile([128, NT, QS], fp8, tag="u_f8")
    if legacy_q:
        q_f8 = sb.tile([128, NT, QS], fp8, tag="q_f8")
        for t in range(NT):
            psq = ps_mm.tile([128, 1024], f32, tag="mm")
            for qc2 in range(4):
                for a in range(2):
                    nc.tensor.matmul(psq[:, qc2 * 256:(qc2 + 1) * 256],
                                     w_q[:, 2 * a:2 * a + 2, t * 128:(t + 1) * 128],
                                     xpair[a][:, :, qc2 * 256:(qc2 + 1) * 256],
                                     start=(a == 0 and qc2 % 2 == 0),
                                     stop=(a == 1), perf_mode=DR)
            if t % 2 == 0:
                nc.scalar.activation(q_f8[:, t, :], psq[:], AF.Identity,
                                     bias=bq2[:, t:t + 1])
            else:
                nc.vector.tensor_scalar_add(q_f8[:, t, :], psq[:],
                                            bq2[:, t:t + 1])
        for t in range(NT):
            psu = ps_mm.tile([128, 1024], f32, tag="mm")
            for qc2 in range(4):
                for a in range(2):
                    nc.tensor.matmul(psu[:, qc2 * 256:(qc2 + 1) * 256],
                                     w_kT[:, 2 * a:2 * a + 2, t * 128:(t + 1) * 128],
                                     q_f8[:, 2 * a:2 * a + 2, qc2 * 256:(qc2 + 1) * 256],
                                     start=(a == 0 and qc2 % 2 == 0),
                                     stop=(a == 1), perf_mode=DR)
            if t % 2 == 0:
                nc.scalar.mul(u_f8[:, t, :], psu[:], sc6_t[t][:])
            else:
                nc.vector.tensor_scalar_mul(u_f8[:, t, :], psu[:], sc6_t[t][:])
    else:
        for t in range(NT):
            psu = ps_mm.tile([128, 1024], f32, tag="mm")
            for qc2 in range(4):
                for a in range(2):
                    nc.tensor.matmul(psu[:, qc2 * 256:(qc2 + 1) * 256],
                                     w_q[:, 2 * a:2 * a + 2, t * 128:(t + 1) * 128],
                                     xpair[a][:, :, qc2 * 256:(qc2 + 1) * 256],
                                     start=(a == 0 and qc2 % 2 == 0),
                                     stop=(a == 1), perf_mode=DR)
            if t % 2 == 0:
                nc.scalar.activation(u_f8[:, t, :], psu[:], AF.Identity,
                                     scale=scale_t[t][:], bias=ub[:, t:t + 1])
            else:
                nc.vector.tensor_scalar(u_f8[:, t, :], psu[:], scale_t[t][:],
                                        ub[:, t:t + 1], op0=ALU.mult,
                                        op1=ALU.add)

    sexp = SEXP_LEGACY if legacy_q else SEXP_FAST

    # ---- phase 2 ---------------------------------------------------
    # v-projection is interleaved into qch 0 (the vt tiles for quad qd are
    # drained two quads before their o-matmuls need them). Quads are
    # software-pipelined with a skew of 2 (o/sums of quad qd-2 are emitted
    # after scores+exp of quad qd) and each qch's scalar tail (reciprocal,
    # o drain, proj, residual) is deferred until two quads into the NEXT
    # qch, so PE/ACT never sit in the serial tail chain.
    vt_f8 = sb.tile([128, N // 128, C], fp8, tag="vt_f8")

    def emit_v(np_, eng):
        psv = ps_mm.tile([128, 2, C], f32, tag="mm")
        for j in range(2):
            nt = 2 * np_ + j
            for co in range(2):
                for a in range(2):
                    nc.tensor.matmul(psv[:, j, co * 256:(co + 1) * 256],
                                     xpair[a][:, :, nt * 128:(nt + 1) * 128],
                                     w_v[:, 2 * a:2 * a + 2, co * 256:(co + 1) * 256],
                                     start=(co == 0 and a == 0), stop=(a == 1),
                                     perf_mode=DR)
        if eng == "dve":
            nc.vector.tensor_copy(vt_f8[:, 2 * np_:2 * np_ + 2, :], psv[:])
        else:
            nc.scalar.copy(vt_f8[:, 2 * np_:2 * np_ + 2, :], psv[:])

    def flush(st, pp_q, qd):
        ob, sums_ps = st
        for a2 in range(2):
            kt0 = qd * 4 + 2 * a2
            first = (qd == 0 and a2 == 0)
            last = (qd == 7 and a2 == 1)
            for ct in range(4):
                nc.tensor.matmul(ob[ct // 2][:, ct % 2, :],
                                 vt_f8[:, kt0:kt0 + 2, ct * 128:(ct + 1) * 128],
                                 pp_q[:, 2 * a2:2 * a2 + 2, :],
                                 start=(first and ct % 2 == 0), stop=last,
                                 perf_mode=DR)
            nc.tensor.matmul(sums_ps[:, 0:QC], ones4[:],
                             pp_q[:, 2 * a2:2 * a2 + 2, :],
                             start=first, stop=last, perf_mode=DR)

    def tail_a(st):
        ob, sums_ps = st
        r_sb = small.tile([1, QC], f32, name="r_sb", tag="r_sb")
        nc.vector.reciprocal(r_sb[:], sums_ps[0:1, 0:QC])
        r_bf = small.tile([1, QC], bf16, name="r_bf", tag="r_bf")
        nc.vector.tensor_copy(r_bf[:], r_sb[:])
        ps_r = ps_mm.tile([128, 1024], f32, name="ps_r", tag="mm")
        nc.tensor.matmul(ps_r[:, 0:QC], ones_row[:], r_bf[:],
                         start=True, stop=True)
        r_bc2 = small.tile([128, 2, QC], f32, name="r_bc2", tag="r_bc2")
        nc.vector.tensor_copy(r_bc2[:, 0, :], ps_r[:, 0:QC])
        nc.vector.tensor_copy(r_bc2[:, 1, :], ps_r[:, 0:QC])
        return r_bc2

    def tail_b(qch, st, r_bc2):
        ob, sums_ps = st
        q0 = qch * QC
        # o drain fuses softmax normalization: o_f8 = ob * r (= 8 * o_norm)
        o_f8 = scr.tile([128, 4, QC], fp8, name="o_f8", tag="o_f8")
        nc.vector.tensor_tensor(o_f8[:, 0:2, :], ob[0][:], r_bc2[:], op=ALU.mult)
        nc.vector.tensor_tensor(o_f8[:, 2:4, :], ob[1][:], r_bc2[:], op=ALU.mult)
        for half in range(2):
            ppb = ps_o.tile([128, 2, QC], f32, name=f"pp{half}", tag=f"ob{half}")
            for tl in range(2):
                t = 2 * half + tl
                for a in range(2):
                    nc.tensor.matmul(ppb[:, tl, :],
                                     w_p[:, 2 * a:2 * a + 2, t * 128:(t + 1) * 128],
                                     o_f8[:, 2 * a:2 * a + 2, :],
                                     start=(tl == 0 and a == 0), stop=(a == 1),
                                     perf_mode=DR)
            outb = outp.tile([128, 2, QC], f32, name="outb", tag="outb")
            nc.vector.scalar_tensor_tensor(
                outb[:], ppb[:], OSC,
                x_sl[:, 2 * half:2 * half + 2, q0:q0 + QC],
                op0=ALU.mult, op1=ALU.add)
            for tl in range(2):
                nc.sync.dma_start(
                    out.ap()[(2 * half + tl) * 128:(2 * half + tl + 1) * 128,
                             q0:q0 + QC],
                    outb[:, tl, :])

    for np_ in range(16):
        emit_v(np_, "dve" if np_ % 2 == 0 else "act")

    sts = {}
    rbc = {}
    fq = []

    def pop_flush():
        fqch, fqd, fp = fq.pop(0)
        if fqch not in sts:
            sts[fqch] = ([ps_o.tile([128, 2, QC], f32, name=f"ob{h}",
                                    tag=f"ob{h}") for h in range(2)],
                         ps_sums.tile([32, 512], f32, name="sums", tag="sums"))
        flush(sts[fqch], fp, fqd)

    for qch in range(QS // QC):
        q0 = qch * QC
        for qd in range(8):
            stq = ps_mm.tile([128, 4, QC], f32, name="stq", tag="mm")
            for j in range(4):
                kt = qd * 4 + j
                for a in range(2):
                    nc.tensor.matmul(stq[:, j, :],
                                     xpair[a][:, :, kt * 128:(kt + 1) * 128],
                                     u_f8[:, 2 * a:2 * a + 2, q0:q0 + QC],
                                     start=(j % 2 == 0 and a == 0), stop=(a == 1),
                                     perf_mode=DR)
            p_q = pq.tile([128, 4, QC], fp8, name="p_q", tag="p")
            nc.scalar.activation(p_q[:], stq[:], AF.Exp, scale=sexp)
            if qch > 0 and qd == 0:
                rbc[qch - 1] = tail_a(sts[qch - 1])
            if qch > 0 and qd == 3:
                tail_b(qch - 1, sts[qch - 1], rbc[qch - 1])
            fq.append((qch, qd, p_q))
            if len(fq) > 3:
                pop_flush()
        if qch == QS // QC - 1:
            while fq:
                pop_flush()
    rbc[3] = tail_a(sts[3])
    tail_b(3, sts[3], rbc[3])


def _make_pools(tc, ctx):
    p = {}
    p["sb"] = ctx.enter_context(tc.tile_pool(name="sb", bufs=1))
    p["scr"] = ctx.enter_context(tc.tile_pool(name="scr", bufs=2))
    p["pq"] = ctx.enter_context(tc.tile_pool(name="pq", bufs=4))
    p["outp"] = ctx.enter_context(tc.tile_pool(name="outp", bufs=2))
    p["small"] = ctx.enter_context(tc.tile_pool(name="small", bufs=1))
    p["ps_mm"] = ctx.enter_context(tc.tile_pool(name="ps_mm", bufs=2, space="PSUM"))
    p["ps_o"] = ctx.enter_context(tc.tile_pool(name="ps_o", bufs=1, space="PSUM"))
    p["ps_sums"] = ctx.enter_context(tc.tile_pool(name="ps_sums", bufs=1, space="PSUM"))
    p["ps_sm"] = ctx.enter_context(tc.tile_pool(name="ps_sm", bufs=1, space="PSUM"))
    return p


def _build(legacy_q=False):
    nc = bacc.Bacc("TRN2", target_bir_lowering=False, debug=False, num_devices=8)
    x8_d = nc.dram_tensor("x8", [C, N], fp8, kind="ExternalInput")
    xsl_d = nc.dram_tensor("xsl", [C, QS], f32, kind="ExternalInput")
    wall_d = nc.dram_tensor("wall", [128, 4 * NT * C], fp8, kind="ExternalInput")
    misc_d = nc.dram_tensor("misc", [128, 672], f32, kind="ExternalInput")
    out = nc.dram_tensor("out", [C, QS], f32, kind="ExternalOutput")
    with tile.TileContext(nc) as tc:
        with contextlib.ExitStack() as ctx:
            p = _make_pools(tc, ctx)
            _emit_body(nc, tc, p, x8_d, xsl_d, wall_d, misc_d, out, legacy_q)
    nc.compile()
    return nc


_NC = {}


def _get_nc(legacy_q=False):
    if legacy_q not in _NC:
        _NC[legacy_q] = _build(legacy_q)
    return _NC[legacy_q]


def kernel(x, gn_w, gn_b, wq, bq, wk, bk, wv, bv, wp, bp):
    x = np.asarray(x, dtype=np.float32)
    B = x.shape[0]
    assert x.shape == (B, C, 64, 64)

    # Fast path (bq == 0): q slot carries M = Wk^T Wq so scores come
    # straight from x. Legacy: q slot = Wq, k slot = Wk untransposed
    # (stationary of u = Wk^T q).
    legacy = bool(np.any(np.asarray(bq, np.float32) != 0.0))
    q_slot = (np.asarray(wk, np.float32).T @ np.asarray(wq, np.float32)
              if not legacy else np.asarray(wq, np.float32))
    wall = np.zeros((128, 4, NT, C), F8)
    for iw, (w, tr) in enumerate(((q_slot, True), (wk, False), (wv, True),
                                 (wp, True))):
        wm = np.asarray(w, np.float32) * SW
        if tr:
            wm = wm.T
        wall[:, iw, :, :] = wm.reshape(NT, 128, C).transpose(1, 0, 2).astype(F8)
    wall = wall.reshape(128, 4 * NT * C)

    misc = np.zeros((128, 672), np.float32)
    for t in range(NT):
        sl = slice(t * 128, (t + 1) * 128)
        misc[:, t] = np.asarray(bq, np.float32)[sl] * SW
        misc[:, 4 + t] = np.asarray(bv, np.float32)[sl] * SW
        misc[:, 8 + t] = np.asarray(bp, np.float32)[sl]
        misc[:, 12 + t] = np.asarray(gn_w, np.float32)[sl]
        misc[:, 16 + t] = np.asarray(gn_b, np.float32)[sl]
    sel8 = np.zeros((128, 8), np.float32)
    for pp_ in range(128):
        sel8[pp_, pp_ // GP] = 1.0
    misc[:, 20:148] = np.concatenate(
        [sel8, np.zeros((128, 120), np.float32)], axis=1)
    misc[0:8, 160:288] = sel8.T

    xf = x.reshape(B, C, N)
    in_maps = []
    for core in range(8):
        b, slab = core // 4, core % 4
        xr = np.roll(xf[b], -QS * slab, axis=1)
        in_maps.append({
            "x8": np.ascontiguousarray(xr).astype(F8),
            "xsl": np.ascontiguousarray(xr[:, 0:QS]),
            "wall": wall, "misc": misc,
        })

    nc = _get_nc(legacy)
    res = bass_utils.run_bass_kernel_spmd(nc, in_maps, core_ids=list(range(8)))

    out = np.empty((B, C, N), np.float32)
    for core in range(8):
        b, slab = core // 4, core % 4
        out[b][:, QS * slab:QS * (slab + 1)] = res.results[core]["out"]
    return out.reshape(B, C, 64, 64)


if __name__ == "__main__":
    rng = np.random.default_rng(0)
    inputs = {
        "x": rng.standard_normal((2, C, 64, 64)).astype(np.float32),
        "gn_w": np.ones(C, np.float32),
        "gn_b": np.zeros(C, np.float32),
    }
    for nm in ("q", "k", "v", "p"):
        inputs[f"w{nm}"] = (rng.standard_normal((C, C)) * 0.02).astype(np.float32)
        inputs[f"b{nm}"] = np.zeros(C, np.float32)
    out = kernel(**inputs)
    print("ran:", out.shape, out.dtype)


# revision 19
# speedup vs baseline: 1.1029x; 1.1029x over previous
"""AttnBlock (GroupNorm -> QKV -> 4096x4096 single-head attention -> proj ->
residual) on 8 TRN2 NeuronCores.

Sharding: data-parallel over batch (B=2) x sequence-parallel over query
positions (4 slabs of 1024). Each core receives the full x[b] (rolled so its
query slab sits at columns 0:1024), computes GroupNorm + v for the whole
image (replicated within the 4-core batch group -> zero collectives), and
attention + projection + residual for its 1024 query columns only.

Precision/structure highlights:
- All heavy matmuls are fp8e4m3 DoubleRow (2x128 contraction / instr at
  0.5 cyc/row), fp32 PSUM.
- k is never materialized: score = (Wk^T q)^T . h, so we compute
  u = Wk^T q_f8 (C x 1024) and use the fp8 x itself as the score
  stationary. The bk term is per-query constant -> softmax invariant ->
  dropped.
- GroupNorm folds: scale -> folded into wq/wv weight tiles on device
  (per-partition) and into the u drain; shift -> tiny N=1 bias-chain
  matmuls (bq2 = 32(Wq shift + bq)) plus a per-channel residual constant
  out += Wp (Wv shift + bv) handled via x_sl. GN stats use a stride-8
  subsample (var estimator err ~1.6%, far below fp8 noise).
- Softmax denominators via a DoubleRow matmul with constant-4.0
  stationary; the o drain fuses the normalization (tensor_tensor with the
  broadcast reciprocal).
"""
import sys
sys.path.insert(0, '/opt/trn_rl_repo')
import contextlib
import numpy as np
import ml_dtypes

import concourse.bass as bass
import concourse.tile as tile
from concourse import mybir, bacc
from concourse import bass_utils

f32 = mybir.dt.float32
bf16 = mybir.dt.bfloat16
fp8 = mybir.dt.float8e4
AF = mybir.ActivationFunctionType
ALU = mybir.AluOpType
DR = mybir.MatmulPerfMode.DoubleRow
F8 = ml_dtypes.float8_e4m3

C = 512          # channels
N = 4096         # positions (64*64)
G = 32           # groupnorm groups
GP = 16          # channels per group
NT = C // 128    # 4 channel partition-tiles
QS = 1024        # query slab per core
QC = 256         # query chunk in attention phase
EPS = 1e-6
SW = 32.0        # weight scale folded into all four fp8 weights
SEXP_LEGACY = 1.0 / (16.0 * float(np.sqrt(C)))
SEXP_FAST = 1.0 / (32.0 * float(np.sqrt(C)))
OSC = 2.0 ** -8  # final pp drain scale (pp = 256 * wp.o_norm)
STRIDE = 8       # GN stats subsample stride
NSAMP = float(GP * (N // STRIDE))        # samples per group


def _emit_body(nc, tc, p, x8_d, xsl_d, wall_d, misc_d, out, legacy_q):
    sb, scr, pq, outp, small = p["sb"], p["scr"], p["pq"], p["outp"], p["small"]
    ps_mm, ps_o, ps_sums, ps_sm = p["ps_mm"], p["ps_o"], p["ps_sums"], p["ps_sm"]

    ones_row = small.tile([1, 128], bf16, tag="ones_row")
    nc.vector.memset(ones_row[:], 1.0)
    ones4 = small.tile([128, 2, 32], fp8, tag="ones4")
    nc.vector.memset(ones4[:], 4.0)
    eps8 = small.tile([8, 1], f32, tag="eps8")
    nc.vector.memset(eps8[:], EPS)
    # warm the abs-rsqrt act table (shares Square/Identity/Copy) while the
    # DMAs run; the exp table is warmed right after rstd below.
    dum = small.tile([1, 16], f32, tag="dum")
    nc.vector.memset(dum[:], 1.0)
    nc.scalar.activation(dum[:], dum[:], AF.Abs_reciprocal_sqrt)

    # ---- phase 0: DMA + GN stats ---------------------------------
    xr = x8_d.ap().rearrange("(t p) n -> p t n", p=128)
    x8a = sb.tile([128, 2, N], fp8, tag="x8a")
    nc.sync.dma_start(x8a[:], xr[:, 0:2, :])
    x8b = sb.tile([128, 2, N], fp8, tag="x8b")
    nc.sync.dma_start(x8b[:], xr[:, 2:4, :])
    misc = sb.tile([128, 672], f32, tag="misc")
    nc.sync.dma_start(misc[:], misc_d.ap())
    wall = sb.tile([128, 4, NT, C], fp8, tag="wall")
    nc.sync.dma_start(wall[:], wall_d.ap())
    xpair = [x8a, x8b]

    w_q, w_kT, w_v, w_p = (wall[:, i, :, :] for i in range(4))
    bq32 = [misc[:, t:t + 1] for t in range(NT)]          # 32*bq
    bv32 = [misc[:, 4 + t:5 + t] for t in range(NT)]      # 32*bv (cout tiles)
    bp_t = [misc[:, 8 + t:9 + t] for t in range(NT)]
    gnw_t = [misc[:, 12 + t:13 + t] for t in range(NT)]
    gnb_t = [misc[:, 16 + t:17 + t] for t in range(NT)]
    sel8p = misc[:, 20:148]        # [128,128] group selector, zero-padded
    sel8T = misc[0:8, 160:288]

    # stats on stride-8 subsample: sum on DVE, sumsq on ACT
    stats2 = small.tile([128, 8], f32, tag="stats2")      # cols 0:4 sum, 4:8 sq
    for t in range(NT):
        samp = xpair[t // 2][:, t % 2, :].rearrange(
            "p (n s) -> p s n", s=STRIDE)[:, 0, :]
        nc.vector.reduce_sum(stats2[:, t:t + 1], samp, axis=mybir.AxisListType.X)
        sqs = scr.tile([128, N // STRIDE], bf16, tag="sqs")
        nc.scalar.activation(sqs[:], samp, AF.Square,
                             accum_out=stats2[:, 4 + t:5 + t])

    # batched GN finalize: one Ln + one Exp for all 32 groups
    ps_all = ps_sm.tile([128, 512], f32, tag="sm")    # one bank for all
    # sel8p is zero-padded to 128 stationary columns so this first matmul
    # covers all 128 partitions: its start=True pending-zeroes the whole
    # bank for every later start=False matmul below.
    ps_g = ps_all[:, 0:8]
    nc.tensor.matmul(ps_g, sel8p, stats2[:], start=True, stop=True)
    mst = small.tile([8, 8], f32, tag="mst")
    nc.scalar.mul(mst[:], ps_g[0:8, :], 1.0 / NSAMP)           # mean t | ex2 t
    var4 = small.tile([8, 4], f32, tag="var4")
    m2 = small.tile([8, 4], f32, tag="m2")
    nc.vector.tensor_tensor(m2[:], mst[:, 0:4], mst[:, 0:4], op=ALU.mult)
    nc.vector.tensor_tensor(var4[:], mst[:, 4:8], m2[:], op=ALU.subtract)
    grp2 = small.tile([8, 2, NT], f32, tag="grp2")        # mean row | rstd row
    nc.vector.tensor_copy(grp2[:, 0, :], mst[:, 0:4])
    nc.scalar.activation(grp2[:, 1, :], var4[:], AF.Abs_reciprocal_sqrt,
                         bias=eps8[:])
    nc.scalar.activation(dum[:], dum[:], AF.Exp)  # warm exp table off-path
    # NOTE: ps_g's start=True zeroed this whole PSUM bank; every later
    # matmul into the ps_sm bank must use start=False (pending-zero) so it
    # does not clobber earlier results that are still being read.
    ps_bc = ps_all[:, 8:16]                               # mean t | rstd t
    nc.tensor.matmul(ps_bc, sel8T, grp2[:].rearrange("p a b -> p (a b)"),
                     start=False, stop=True)
    # scale/shift per tile + derived columns
    scale_t, sc6_t = [], []
    sh8 = small.tile([128, NT], fp8, tag="sh8")           # 128*shift, fp8
    for t in range(NT):
        sc = small.tile([128, 1], f32, tag=f"scale{t}")
        nc.vector.tensor_tensor(sc[:], gnw_t[t], ps_bc[:, 4 + t:5 + t],
                                op=ALU.mult)
        sc6 = small.tile([128, 1], f32, tag=f"sc6{t}")
        nc.vector.tensor_scalar_mul(sc6[:], sc[:], 2.0 ** -6)
        nsc = small.tile([128, 1], f32, tag=f"nscale{t}")
        nc.vector.tensor_scalar_mul(nsc[:], sc[:], -1.0)
        sh = small.tile([128, 1], f32, tag=f"shift{t}")
        nc.vector.scalar_tensor_tensor(sh[:], ps_bc[:, t:t + 1], nsc[:],
                                       gnb_t[t], op0=ALU.mult, op1=ALU.add)
        nc.vector.tensor_scalar_mul(sh8[:, t:t + 1], sh[:], 128.0)
        scale_t.append(sc)
        sc6_t.append(sc6)

    # bias chains (N=1 fp8 matmuls, before the in-place weight folds):
    # bq2 = 32(Wq shift + bq);  vc = 2^5 (Wv shift + bv);
    # badd = Wp vc / 2^10  (-> residual constant Wp(Wv shift + bv))
    ps_bq = ps_all[:, 16:20]
    ps_vc = ps_all[:, 20:24]
    for t in range(NT):
        for s in range(NT):
            nc.tensor.matmul(ps_bq[:, t:t + 1],
                             w_q[:, s, t * 128:(t + 1) * 128], sh8[:, s:s + 1],
                             start=False, stop=(s == 3))
            nc.tensor.matmul(ps_vc[:, t:t + 1],
                             w_v[:, s, t * 128:(t + 1) * 128], sh8[:, s:s + 1],
                             start=False, stop=(s == 3))
    if legacy_q:
        bq2 = small.tile([128, 4], f32, tag="bq2")
        for t in range(NT):
            nc.vector.tensor_scalar(bq2[:, t:t + 1], ps_bq[:, t:t + 1],
                                    2.0 ** -7, bq32[t], op0=ALU.mult,
                                    op1=ALU.add)
    else:
        # ub = scale o (M shift) * 32  (chain psum = 4096 * M shift)
        ub = small.tile([128, 4], f32, tag="ub")
        for t in range(NT):
            nc.vector.tensor_scalar(ub[:, t:t + 1], ps_bq[:, t:t + 1],
                                    scale_t[t][:], 2.0 ** -7, op0=ALU.mult,
                                    op1=ALU.mult)
    vc8 = small.tile([128, 4], fp8, tag="vc8")
    for t in range(NT):
        nc.scalar.activation(vc8[:, t:t + 1], ps_vc[:, t:t + 1], AF.Identity,
                             scale=2.0 ** -7, bias=bv32[t])
    ps_t2 = ps_all[:, 24:28]
    for t in range(NT):
        for s in range(NT):
            nc.tensor.matmul(ps_t2[:, t:t + 1],
                             w_p[:, s, t * 128:(t + 1) * 128], vc8[:, s:s + 1],
                             start=False, stop=(s == 3))
    badd = small.tile([128, 4], f32, tag="badd")
    for t in range(NT):
        nc.vector.tensor_scalar(badd[:, t:t + 1], ps_t2[:, t:t + 1],
                                2.0 ** -10, bp_t[t], op0=ALU.mult, op1=ALU.add)

    # fold GN scale into wq / wv (per-partition, in place; wq on DVE so
    # the u projection can start while Pool folds wv)
    for s in range(NT):
        nc.vector.tensor_scalar_mul(w_q[:, s, :], w_q[:, s, :], scale_t[s][:])
    for s in range(NT):
        nc.gpsimd.tensor_scalar_mul(w_v[:, s, :], w_v[:, s, :], scale_t[s][:])

    # residual slab + per-channel constant (bp + Wp(Wv shift + bv))
    x_sl = sb.tile([128, NT, QS], f32, tag="x_sl")
    nc.sync.dma_start(x_sl[:], xsl_d.ap().rearrange("(t p) n -> p t n", p=128))
    for t in range(NT):
        nc.gpsimd.tensor_scalar_add(x_sl[:, t, :], x_sl[:, t, :],
                                    badd[:, t:t + 1])

    # ---- phase 1: u (drains alternate ACT/DVE) -------------------
    # Fast path (bq == 0): host sends M = Wk^T Wq in the q slot, so
    # u = scale o (M (scale o x + shift)) comes straight from x with no q
    # projection; the shift term lands in the u-drain bias via the chain.
    # Legacy path (bq != 0): q = Wq h + bq is materialized, then
    # u = Wk^T q (the bq^T k score term is not softmax-invariant).
    u_f8 = sb.tile([128, NT, QS], fp8, tag="u_f8")
    if legacy_q:
        q_f8 = sb.tile([128, NT, QS], fp8, tag="q_f8")
        for t in range(NT):
            psq = ps_mm.tile([128, 1024], f32, tag="mm")
            for qc2 in range(4):
                for a in range(2):
                    nc.tensor.matmul(psq[:, qc2 * 256:(qc2 + 1) * 256],
                                     w_q[:, 2 * a:2 * a + 2, t * 128:(t + 1) * 128],
                                     xpair[a][:, :, qc2 * 256:(qc2 + 1) * 256],
                                     start=(a == 0 and qc2 % 2 == 0),
                                     stop=(a == 1), perf_mode=DR)
            if t % 2 == 0:
                nc.scalar.activation(q_f8[:, t, :], psq[:], AF.Identity,
                                     bias=bq2[:, t:t + 1])
            else:
                nc.vector.tensor_scalar_add(q_f8[:, t, :], psq[:],
                                            bq2[:, t:t + 1])
        for t in range(NT):
            psu = ps_mm.tile([128, 1024], f32, tag="mm")
            for qc2 in range(4):
                for a in range(2):
                    nc.tensor.matmul(psu[:, qc2 * 256:(qc2 + 1) * 256],
                                     w_kT[:, 2 * a:2 * a + 2, t * 128:(t + 1) * 128],
                                     q_f8[:, 2 * a:2 * a + 2, qc2 * 256:(qc2 + 1) * 256],
                                     start=(a == 0 and qc2 % 2 == 0),
                                     stop=(a == 1), perf_mode=DR)
            if t % 2 == 0:
                nc.scalar.mul(u_f8[:, t, :], psu[:], sc6_t[t][:])
            else:
                nc.vector.tensor_scalar_mul(u_f8[:, t, :], psu[:], sc6_t[t][:])
    else:
        for t in range(NT):
            psu = ps_mm.tile([128, 1024], f32, tag="mm")
            for qc2 in range(4):
                for a in range(2):
                    nc.tensor.matmul(psu[:, qc2 * 256:(qc2 + 1) * 256],
                                     w_q[:, 2 * a:2 * a + 2, t * 128:(t + 1) * 128],
                                     xpair[a][:, :, qc2 * 256:(qc2 + 1) * 256],
                                     start=(a == 0 and qc2 % 2 == 0),
                                     stop=(a == 1), perf_mode=DR)
            if t % 2 == 0:
                nc.scalar.activation(u_f8[:, t, :], psu[:], AF.Identity,
                                     scale=scale_t[t][:], bias=ub[:, t:t + 1])
            else:
                nc.vector.tensor_scalar(u_f8[:, t, :], psu[:], scale_t[t][:],
                                        ub[:, t:t + 1], op0=ALU.mult,
                                        op1=ALU.add)

    sexp = SEXP_LEGACY if legacy_q else SEXP_FAST

    # ---- phase 2 ---------------------------------------------------
    # v-projection is interleaved into qch 0 (the vt tiles for quad qd are
    # drained two quads before their o-matmuls need them). Quads are
    # software-pipelined with a skew of 2 (o/sums of quad qd-2 are emitted
    # after scores+exp of quad qd) and each qch's scalar tail (reciprocal,
    # o drain, proj, residual) is deferred until two quads into the NEXT
    # qch, so PE/ACT never sit in the serial tail chain.
    vt_f8 = sb.tile([128, N // 128, C], fp8, tag="vt_f8")

    def emit_v(np_, eng):
        psv = ps_mm.tile([128, 2, C], f32, tag="mm")
        for j in range(2):
            nt = 2 * np_ + j
            for co in range(2):
                for a in range(2):
                    nc.tensor.matmul(psv[:, j, co * 256:(co + 1) * 256],
                                     xpair[a][:, :, nt * 128:(nt + 1) * 128],
                                     w_v[:, 2 * a:2 * a + 2, co * 256:(co + 1) * 256],
                                     start=(co == 0 and a == 0), stop=(a == 1),
                                     perf_mode=DR)
        if eng == "dve":
            nc.vector.tensor_copy(vt_f8[:, 2 * np_:2 * np_ + 2, :], psv[:])
        else:
            nc.scalar.copy(vt_f8[:, 2 * np_:2 * np_ + 2, :], psv[:])

    def flush(st, pp_q, qd):
        ob, sums_ps = st
        for a2 in range(2):
            kt0 = qd * 4 + 2 * a2
            first = (qd == 0 and a2 == 0)
            last = (qd == 7 and a2 == 1)
            for ct in range(4):
                nc.tensor.matmul(ob[ct // 2][:, ct % 2, :],
                                 vt_f8[:, kt0:kt0 + 2, ct * 128:(ct + 1) * 128],
                                 pp_q[:, 2 * a2:2 * a2 + 2, :],
                                 start=(first and ct % 2 == 0), stop=last,
                                 perf_mode=DR)
            nc.tensor.matmul(sums_ps[:, 0:QC], ones4[:],
                             pp_q[:, 2 * a2:2 * a2 + 2, :],
                             start=first, stop=last, perf_mode=DR)

    def tail_a(st):
        ob, sums_ps = st
        r_sb = small.tile([1, QC], f32, name="r_sb", tag="r_sb")
        nc.vector.reciprocal(r_sb[:], sums_ps[0:1, 0:QC])
        r_bf = small.tile([1, QC], bf16, name="r_bf", tag="r_bf")
        nc.vector.tensor_copy(r_bf[:], r_sb[:])
        ps_r = ps_mm.tile([128, 1024], f32, name="ps_r", tag="mm")
        nc.tensor.matmul(ps_r[:, 0:QC], ones_row[:], r_bf[:],
                         start=True, stop=True)
        r_bc2 = small.tile([128, 2, QC], f32, name="r_bc2", tag="r_bc2")
        nc.vector.tensor_copy(r_bc2[:, 0, :], ps_r[:, 0:QC])
        nc.vector.tensor_copy(r_bc2[:, 1, :], ps_r[:, 0:QC])
        return r_bc2

    def tail_b(qch, st, r_bc2):
        ob, sums_ps = st
        q0 = qch * QC
        # o drain fuses softmax normalization: o_f8 = ob * r (= 8 * o_norm)
        o_f8 = scr.tile([128, 4, QC], fp8, name="o_f8", tag="o_f8")
        nc.vector.tensor_tensor(o_f8[:, 0:2, :], ob[0][:], r_bc2[:], op=ALU.mult)
        nc.vector.tensor_tensor(o_f8[:, 2:4, :], ob[1][:], r_bc2[:], op=ALU.mult)
        for half in range(2):
            ppb = ps_o.tile([128, 2, QC], f32, name=f"pp{half}", tag=f"ob{half}")
            for tl in range(2):
                t = 2 * half + tl
                for a in range(2):
                    nc.tensor.matmul(ppb[:, tl, :],
                                     w_p[:, 2 * a:2 * a + 2, t * 128:(t + 1) * 128],
                                     o_f8[:, 2 * a:2 * a + 2, :],
                                     start=(tl == 0 and a == 0), stop=(a == 1),
                                     perf_mode=DR)
            outb = outp.tile([128, 2, QC], f32, name="outb", tag="outb")
            nc.vector.scalar_tensor_tensor(
                outb[:], ppb[:], OSC,
                x_sl[:, 2 * half:2 * half + 2, q0:q0 + QC],
                op0=ALU.mult, op1=ALU.add)
            for tl in range(2):
                nc.sync.dma_start(
                    out.ap()[(2 * half + tl) * 128:(2 * half + tl + 1) * 128,
                             q0:q0 + QC],
                    outb[:, tl, :])

    for np_ in range(16):
        emit_v(np_, "dve" if np_ % 2 == 0 else "act")

    sts = {}
    rbc = {}
    fq = []

    def pop_flush():
        fqch, fqd, fp = fq.pop(0)
        if fqch not in sts:
            sts[fqch] = ([ps_o.tile([128, 2, QC], f32, name=f"ob{h}",
                                    tag=f"ob{h}") for h in range(2)],
                         ps_sums.tile([32, 512], f32, name="sums", tag="sums"))
        flush(sts[fqch], fp, fqd)

    for qch in range(QS // QC):
        q0 = qch * QC
        for qd in range(8):
            stq = ps_mm.tile([128, 4, QC], f32, name="stq", tag="mm")
            for j in range(4):
                kt = qd * 4 + j
                for a in range(2):
                    nc.tensor.matmul(stq[:, j, :],
                                     xpair[a][:, :, kt * 128:(kt + 1) * 128],
                                     u_f8[:, 2 * a:2 * a + 2, q0:q0 + QC],
                                     start=(j % 2 == 0 and a == 0), stop=(a == 1),
                                     perf_mode=DR)
            p_q = pq.tile([128, 4, QC], fp8, name="p_q", tag="p")
            nc.scalar.activation(p_q[:], stq[:], AF.Exp, scale=sexp)
            if qch > 0 and qd == 0:
                rbc[qch - 1] = tail_a(sts[qch - 1])
            if qch > 0 and qd == 3:
                tail_b(qch - 1, sts[qch - 1], rbc[qch - 1])
            fq.append((qch, qd, p_q))
            if len(fq) > 3:
                pop_flush()
        if qch == QS // QC - 1:
            while fq:
                pop_flush()
    rbc[3] = tail_a(sts[3])
    tail_b(3, sts[3], rbc[3])


def _make_pools(tc, ctx):
    p = {}
    p["sb"] = ctx.enter_context(tc.tile_pool(name="sb", bufs=1))
    p["scr"] = ctx.enter_context(tc.tile_pool(name="scr", bufs=2))
    p["pq"] = ctx.enter_context(tc.tile_pool(name="pq", bufs=4))
    p["outp"] = ctx.enter_context(tc.tile_pool(name="outp", bufs=2))
    p["small"] = ctx.enter_context(tc.tile_pool(name="small", bufs=1))
    p["ps_mm"] = ctx.enter_context(tc.tile_pool(name="ps_mm", bufs=2, space="PSUM"))
    p["ps_o"] = ctx.enter_context(tc.tile_pool(name="ps_o", bufs=1, space="PSUM"))
    p["ps_sums"] = ctx.enter_context(tc.tile_pool(name="ps_sums", bufs=1, space="PSUM"))
    p["ps_sm"] = ctx.enter_context(tc.tile_pool(name="ps_sm", bufs=1, space="PSUM"))
    return p


def _build(legacy_q=False):
    nc = bacc.Bacc("TRN2", target_bir_lowering=False, debug=False, num_devices=8)
    x8_d = nc.dram_tensor("x8", [C, N], fp8, kind="ExternalInput")
    xsl_d = nc.dram_tensor("xsl", [C, QS], f32, kind="ExternalInput")
    wall_d = nc.dram_tensor("wall", [128, 4 * NT * C], fp8, kind="ExternalInput")
    misc_d = nc.dram_tensor("misc", [128, 672], f32, kind="ExternalInput")
    out = nc.dram_tensor("out", [C, QS], f32, kind="ExternalOutput")
    with tile.TileContext(nc) as tc:
        with contextlib.ExitStack() as ctx:
            p = _make_pools(tc, ctx)
            _emit_body(nc, tc, p, x8_d, xsl_d, wall_d, misc_d, out, legacy_q)
    nc.compile()
    return nc


_NC = {}


def _get_nc(legacy_q=False):
    if legacy_q not in _NC:
        _NC[legacy_q] = _build(legacy_q)
    return _NC[legacy_q]


def kernel(x, gn_w, gn_b, wq, bq, wk, bk, wv, bv, wp, bp):
    x = np.asarray(x, dtype=np.float32)
    B = x.shape[0]
    assert x.shape == (B, C, 64, 64)

    # Fast path (bq == 0): q slot carries M = Wk^T Wq so scores come
    # straight from x. Legacy: q slot = Wq, k slot = Wk untransposed
    # (stationary of u = Wk^T q).
    legacy = bool(np.any(np.asarray(bq, np.float32) != 0.0))
    q_slot = (np.asarray(wk, np.float32).T @ np.asarray(wq, np.float32)
              if not legacy else np.asarray(wq, np.float32))
    wall = np.zeros((128, 4, NT, C), F8)
    for iw, (w, tr) in enumerate(((q_slot, True), (wk, False), (wv, True),
                                 (wp, True))):
        wm = np.asarray(w, np.float32) * SW
        if tr:
            wm = wm.T
        wall[:, iw, :, :] = wm.reshape(NT, 128, C).transpose(1, 0, 2).astype(F8)
    wall = wall.reshape(128, 4 * NT * C)

    misc = np.zeros((128, 672), np.float32)
    for t in range(NT):
        sl = slice(t * 128, (t + 1) * 128)
        misc[:, t] = np.asarray(bq, np.float32)[sl] * SW
        misc[:, 4 + t] = np.asarray(bv, np.float32)[sl] * SW
        misc[:, 8 + t] = np.asarray(bp, np.float32)[sl]
        misc[:, 12 + t] = np.asarray(gn_w, np.float32)[sl]
        misc[:, 16 + t] = np.asarray(gn_b, np.float32)[sl]
    sel8 = np.zeros((128, 8), np.float32)
    for pp_ in range(128):
        sel8[pp_, pp_ // GP] = 1.0
    misc[:, 20:148] = np.concatenate(
        [sel8, np.zeros((128, 120), np.float32)], axis=1)
    misc[0:8, 160:288] = sel8.T

    xf = x.reshape(B, C, N)
    in_maps = []
    for core in range(8):
        b, slab = core // 4, core % 4
        xr = np.roll(xf[b], -QS * slab, axis=1)
        in_maps.append({
            "x8": np.ascontiguousarray(xr).astype(F8),
            "xsl": np.ascontiguousarray(xr[:, 0:QS]),
            "wall": wall, "misc": misc,
        })

    nc = _get_nc(legacy)
    res = bass_utils.run_bass_kernel_spmd(nc, in_maps, core_ids=list(range(8)))

    out = np.empty((B, C, N), np.float32)
    for core in range(8):
        b, slab = core // 4, core % 4
        out[b][:, QS * slab:QS * (slab + 1)] = res.results[core]["out"]
    return out.reshape(B, C, 64, 64)


if __name__ == "__main__":
    rng = np.random.default_rng(0)
    inputs = {
        "x": rng.standard_normal((2, C, 64, 64)).astype(np.float32),
        "gn_w": np.ones(C, np.float32),
        "gn_b": np.zeros(C, np.float32),
    }
    for nm in ("q", "k", "v", "p"):
        inputs[f"w{nm}"] = (rng.standard_normal((C, C)) * 0.02).astype(np.float32)
        inputs[f"b{nm}"] = np.zeros(C, np.float32)
    out = kernel(**inputs)
    print("ran:", out.shape, out.dtype)
